# revision 15
# baseline (speedup 1.0000x reference)
"""Tropical (min-plus) matmul kernel for Trainium2, SPMD over 8 NeuronCores.

Computes out[b, j] = min_i (X[b, i] + W[j, i]) with B=1024, IN=OUT=512, fp32.

Sharding: data-parallel over batch — core c handles X rows [128c, 128(c+1)),
W replicated (matches the 1MB-weight replication hint).

Per-core pipeline (raw Bass, explicit semaphores):
  PE   : for each i, a K=3 matmul of an all-ones [3,128] stationary operand
         against the 3 bf16 limbs of W^T row i reconstructs W[j,i] broadcast
         over all 128 batch partitions into a PSUM bank — exact fp32 (the
         limbs sum exactly to the fp32 weight).
  ACT  : s_i = psum_bank + X[:, i] (Identity activation with per-partition
         bias; one fp32 round-to-nearest add, bit-matching the reference).
  DVE  : acc = min(acc, s_i) — in-place tensor_tensor min chain.
The three engines run as a software pipeline over an 8-bank PSUM ring and a
16-slot SBUF ring. Hardware allows at most one attached wait per compute
instruction and none on in-place ops, so in-place consumers use standalone
engine waits.
"""

import numpy as np
import ml_dtypes

import concourse.bass as bass
import concourse.mybir as mybir
from concourse.bass_utils import run_bass_kernel_spmd

B, IN, OUT = 1024, 512, 512
NCORES = 8
BLOC = B // NCORES  # batch rows per core = 128
IB = 8  # W^T limb chunks
IR = IN // IB  # i's per chunk = 64
SRING = 16  # SBUF s-tile ring slots
ACC_INIT = 1.0e30

_PROGRAM = None


def _build_program():
    nc = bass.Bass()
    x_in = nc.declare_dram_parameter("Xc", [BLOC, IN], mybir.dt.float32, isOutput=False)
    wtl_in = nc.declare_dram_parameter(
        "WTL", [3 * IB, IR * OUT], mybir.dt.bfloat16, isOutput=False
    )
    out_t = nc.declare_dram_parameter("OUTC", [BLOC, OUT], mybir.dt.float32, isOutput=True)

    with (
        nc.sbuf_tensor([BLOC, IN], mybir.dt.float32) as x_sb,
        nc.sbuf_tensor([3, 2, IR * OUT], mybir.dt.bfloat16) as wc,
        nc.sbuf_tensor([3, BLOC], mybir.dt.bfloat16) as ones_sb,
        nc.sbuf_tensor([BLOC, SRING, OUT], mybir.dt.float32) as sring,
        nc.sbuf_tensor([BLOC, OUT], mybir.dt.float32) as acc,
        nc.psum_tensor([BLOC, 8, OUT], mybir.dt.float32) as banks,
        nc.semaphore("dma_sem") as dma_sem,
        nc.semaphore("out_sem") as out_sem,
        nc.semaphore("wdma_sem") as wdma_sem,
        nc.semaphore("init_sem") as init_sem,
        nc.semaphore("pe_sem") as pe_sem,
        nc.semaphore("act_sem") as act_sem,
        nc.semaphore("dve_sem") as dve_sem,
        nc.Block() as blk,
    ):

        @blk.sync
        def _(sync):
            sync.dma_start(out=x_sb[:], in_=x_in[:, :]).then_inc(dma_sem, 16)
            for g in range(IB):
                if g >= 1:
                    # serialize chunk DMAs: completions of one semaphore can
                    # reorder, so a waiter on 16*(g+1) must imply all earlier
                    # chunks landed
                    sync.wait_ge(wdma_sem, 16 * g)
                if g >= 2:
                    # slot g%2 is free once PE finished chunk g-2
                    sync.wait_ge(pe_sem, (g - 1) * IR)
                sync.dma_start(
                    out=wc[:, g % 2, :], in_=wtl_in[3 * g : 3 * g + 3, :]
                ).then_inc(wdma_sem, 16)
            sync.wait_ge(dve_sem, IN + 1)
            sync.dma_start(out=out_t[:, :], in_=acc[:]).then_inc(out_sem, 16)

        @blk.vector
        def _(vector):
            nc.vector.memset(ones_sb[:], 1.0).then_inc(init_sem, 1)
            # dve_sem counts: 1 (acc memset) + one per TT. The self-wait per
            # iteration orders each in-place TT after the previous write's
            # completion (engines don't guarantee write visibility by program
            # order alone).
            nc.vector.memset(acc[:], ACC_INIT).then_inc(dve_sem, 1)
            for i in range(IN):
                vector.wait_ge(act_sem, i + 1)
                vector.wait_ge(dve_sem, i + 1)
                nc.vector.tensor_tensor(
                    acc[:], acc[:], sring[:, i % SRING, :], mybir.AluOpType.min
                ).then_inc(dve_sem, 1)

        @blk.scalar
        def _(scalar):
            scalar.wait_ge(dma_sem, 16)  # x_sb loaded (bias reads)
            for i in range(IN):
                if i >= SRING and i % 8 == 0:
                    # s-ring slots for [i, i+8) free once DVE consumed i-9
                    # (+1: dve_sem also counts the acc memset)
                    scalar.wait_ge(dve_sem, i + 8 - SRING + 1)
                ins = nc.scalar.activation(
                    sring[:, i % SRING, :],
                    banks[:, i % 8, :],
                    mybir.ActivationFunctionType.Identity,
                    bias=x_sb[:, i : i + 1],
                    scale=1.0,
                )
                ins._wait_ge(pe_sem, i + 1)
                ins.then_inc(act_sem, 1)

        @blk.tensor
        def _(tensor):
            tensor.wait_ge(init_sem, 1)  # ones memset
            for g in range(IB):
                tensor.wait_ge(wdma_sem, 16 * (g + 1))
                for r in range(IR):
                    i = g * IR + r
                    ins = nc.tensor.matmul(
                        banks[:, i % 8, :],
                        ones_sb[:],
                        wc[:, g % 2, r * OUT : (r + 1) * OUT],
                        start=True,
                        stop=True,
                    )
                    if i >= 8:
                        # bank slot reused once ACT consumed i-8
                        ins._wait_ge(act_sem, i - 7)
                    ins.then_inc(pe_sem, 1)

    return nc


def _w_limbs(W: np.ndarray) -> np.ndarray:
    """Split W^T into 3 bf16 limbs (exact fp32 reconstruction), laid out as
    [3*IB partitions, IR*OUT] so partition 3g+c holds limb c of i-block g."""
    WT = np.ascontiguousarray(W.T.astype(np.float32))  # [IN, OUT] = [i, j]
    l0 = WT.astype(ml_dtypes.bfloat16)
    r1 = WT - l0.astype(np.float32)
    l1 = r1.astype(ml_dtypes.bfloat16)
    r2 = r1 - l1.astype(np.float32)
    l2 = r2.astype(ml_dtypes.bfloat16)
    wtl = np.zeros((3 * IB, IR * OUT), dtype=ml_dtypes.bfloat16)
    for g in range(IB):
        blk = slice(g * IR, (g + 1) * IR)
        wtl[3 * g + 0, :] = l0[blk, :].reshape(-1)
        wtl[3 * g + 1, :] = l1[blk, :].reshape(-1)
        wtl[3 * g + 2, :] = l2[blk, :].reshape(-1)
    return wtl


def _run(X: np.ndarray, W: np.ndarray, trace: bool = False, **kwargs):
    global _PROGRAM
    X = np.asarray(X, dtype=np.float32)
    W = np.asarray(W, dtype=np.float32)
    assert X.shape == (B, IN) and W.shape == (OUT, IN)

    if _PROGRAM is None:
        _PROGRAM = _build_program()

    wtl = _w_limbs(W)
    in_maps = [
        {"Xc": np.ascontiguousarray(X[c * BLOC : (c + 1) * BLOC]), "WTL": wtl}
        for c in range(NCORES)
    ]
    res = run_bass_kernel_spmd(
        _PROGRAM, in_maps, list(range(NCORES)), trace=trace, **kwargs
    )
    out = np.concatenate([res.results[c]["OUTC"] for c in range(NCORES)], axis=0)
    return out.astype(np.float32), res


def kernel(X: np.ndarray, W: np.ndarray) -> np.ndarray:
    return _run(X, W)[0]
